# revision 34
# baseline (speedup 1.0000x reference)
"""Multi-Query Attention kernel for 8x TRN2 NeuronCores (Bass/Tile).

Problem: x[B=2, L=2048, D=2048], Wq[2048,2048], Wk/Wv[128,2048] (MQA: one
shared K/V head), 16 query heads of dim 128.

Sharding: core c in [0,8): batch b = c//4, head-group g = c%4 (4 heads,
i.e. q-channels [512g, 512g+512)). K/V replicated per core.

v4 schedule ("DMA-paced front + merged supersteps"): the baseline ran all
projections then all attention; the attention phase was ACT(exp)-paced, so
the PE idled at every pass boundary and each stall re-triggered the PE
p-state ramp.  This version keeps the PE continuously busy:

  1. l-tile 0 projects K, V AND all four Q heads (6 matmuls/chunk — the
     same cadence at which x/w chunks stream in from HBM, so the cold-start
     DMA latency is fully hidden).  l-tiles 1-3 then project K/V only at
     full PE speed from the by-then-resident x.  x stays in SBUF
     (64KB/partition) so the later Q projections re-read it for free.
  2. Supersteps t=0..3: attention passes (lq=t, head-pairs 0,1) emit their
     scores/AV matmuls interleaved with Q l-tile t+1's projection matmuls
     as PE filler.  Per 128-key block the PE does 6 matmuls (~1.3us) vs
     ACT's one 1.0us exp, so the exp pipeline (and the DVE denominator
     adds) hide completely under PE work.
  3. A pass's softmax tail (ones-matmul partition-reduce of the DVE-summed
     denominator -> fast reciprocal -> normalize multiply -> output DMA) is
     emitted inside the NEXT pass's boundary, after the next scores have
     been issued, so the in-order PE never waits on the exp->add chain.
     The AV accumulator is drained PSUM->SBUF by the ACT right after the
     last AV matmul, freeing its 2 PSUM banks for the next pass (PSUM in
     supersteps: 2 Q-projection banks + 2x2 scores banks + 2 AV banks = 8).
  4. DMA issue is spread: SP carries the l-tile-0 x quarters + weights
     (2-3 issues per chunk, matched to the 6-matmul chunk cadence), the
     gpsimd DGE carries the x columns for l-tiles 1-3, ACT only the tiny
     biases.  Output DMAs split in 4 (8 for the final pass) so the last
     transfer isn't serialized on one queue.

Precision (identical math to the HW-verified baseline, rel_err ~2.0e-3 vs
the 2e-2 budget): x/W stream fp16, projections fp16 x fp16 -> fp32 PSUM,
Q/K kept fp16, exp output and V bf16 (DVE accumulates the denominator at
its 2x 16-bit rate), everything normalized in fp32.
"""

import itertools
from contextlib import ExitStack

import numpy as np

import concourse.bass as bass
import concourse.tile as tile
from concourse import bacc, masks, mybir
from concourse.bass_utils import run_bass_kernel_spmd

F32 = mybir.dt.float32
BF16 = mybir.dt.bfloat16
F16 = mybir.dt.float16
AF = mybir.ActivationFunctionType

B = 2
L = 2048
D = 2048  # d_model (contraction dim of projections)
HD = 128  # head dim
NH = 4  # heads per core
QC = NH * HD  # q-channels per core = 512
DC = D // 128  # d-model chunks of 128 = 16
NLT = 4  # l tiles of 512
LKT = L // 128  # lk blocks of 128 = 16
N_CORES = 8
SCALE = 1.0 / float(np.sqrt(HD))


def build_kernel(ctx: ExitStack, tc: tile.TileContext, xT, wT, bq, bk, bv, outT):
    nc = tc.nc

    persist = ctx.enter_context(tc.tile_pool(name="persist", bufs=1))
    x_sb = [persist.tile([128, L], F16, tag=f"x{k}", name=f"x{k}") for k in range(DC)]
    # w chunk: [Wq h0..h3 | Wk | Wv].T slice, one [128, 768] tile per d-chunk
    w_ch = [persist.tile([128, QC + 2 * HD], F16, tag=f"wc{k}", name=f"wc{k}") for k in range(DC)]
    qT = [persist.tile([128, L], F16, tag=f"qT{h}", name=f"qT{h}") for h in range(NH)]  # [d, l]
    kT = persist.tile([128, L], F16, tag="kT", name="kT")  # [d, l]
    vT = [persist.tile([128, 512], BF16, tag=f"vT{t}", name=f"vT{t}") for t in range(NLT)]
    vN = persist.tile([128, L], BF16, tag="vN", name="vN")  # block j: [:, 128j:+128] = V[128j:+128, :]
    ones_bf = persist.tile([128, 128], BF16, tag="ones_bf", name="ones_bf")
    ident = persist.tile([128, 128], BF16, tag="ident", name="ident")
    bq_sb = persist.tile([128, NH], F32, tag="bq", name="bq")
    bk_sb = persist.tile([128, 1], F32, tag="bk", name="bk")
    bv_sb = persist.tile([128, 1], F32, tag="bv", name="bv")

    nc.vector.memset(ones_bf[:], 1.0)
    nc.scalar.dma_start(out=bq_sb[:], in_=bq)
    nc.scalar.dma_start(out=bk_sb[:], in_=bk)
    nc.scalar.dma_start(out=bv_sb[:], in_=bv)

    # ---------------- DMA plan ----------------
    # Everything rides the SP queue in strict priority order: per-queue
    # bandwidth is only ~17GB/s (aggregate ~270GB/s over 16 queues), so the
    # front is DMA-limited and lower-priority pieces must not race the
    # l-tile-0 stream.  Chunk k's pieces are issued so they land just
    # before its 6 l-tile-0 matmuls; the first two chunks split into 64KB
    # pieces to cut the cold-start transfer latency.
    for k in range(DC):
        if k < 2:
            nc.sync.dma_start(out=x_sb[k][:, 0:256], in_=xT[k * 128:(k + 1) * 128, 0:256])
            nc.sync.dma_start(out=x_sb[k][:, 256:512], in_=xT[k * 128:(k + 1) * 128, 256:512])
            nc.sync.dma_start(out=w_ch[k][:, QC:], in_=wT[k * 128:(k + 1) * 128, QC:])
            nc.sync.dma_start(out=w_ch[k][:, 0:256], in_=wT[k * 128:(k + 1) * 128, 0:256])
            nc.sync.dma_start(out=w_ch[k][:, 256:QC], in_=wT[k * 128:(k + 1) * 128, 256:QC])
        elif k < 4:
            nc.sync.dma_start(out=x_sb[k][:, 0:512], in_=xT[k * 128:(k + 1) * 128, 0:512])
            nc.sync.dma_start(out=w_ch[k][:, QC:], in_=wT[k * 128:(k + 1) * 128, QC:])
            nc.sync.dma_start(out=w_ch[k][:, 0:QC], in_=wT[k * 128:(k + 1) * 128, 0:QC])
        else:
            nc.sync.dma_start(out=x_sb[k][:, 0:512], in_=xT[k * 128:(k + 1) * 128, 0:512])
            nc.sync.dma_start(out=w_ch[k][:], in_=wT[k * 128:(k + 1) * 128, :])
    # x columns for l-tiles 1..3, in consumption order behind the l-tile-0
    # stream
    for c0 in (512, 1024, 1536):
        for k in range(DC):
            nc.sync.dma_start(
                out=x_sb[k][:, c0:c0 + 512], in_=xT[k * 128:(k + 1) * 128, c0:c0 + 512]
            )
    # identity only needed by the V transposes (~55us in)
    masks.make_identity(nc, ident[:])

    # ---------------- front ----------------
    # l-tile 0: K, V and all four Q heads (6 matmuls/chunk — matches the
    # HBM arrival rate, so the DMA-limited region is fully PE-utilized).
    # No standalone warmup phase: that would sit behind the DVE's ~7.5us
    # engine-startup (ones memset) while the first x/w pieces land at
    # ~4.5us.  Instead the first matmul starts the moment chunk 0 lands,
    # and p-state warmup matmuls are FILLER inside the DMA-paced sweep so
    # the PE stays continuously busy through the arrival gaps.
    # l-tiles 1-3: K/V only, at full PE speed from resident x.  Q slices
    # 1-3 become superstep filler.
    with tc.tile_pool(name="kvp", bufs=1, space="PSUM") as kvp:
        with (
            tc.tile_pool(name="qf", bufs=1, space="PSUM") as qf,
            tc.tile_pool(name="warm", bufs=1, space="PSUM") as wp,
        ):
            pw = wp.tile([128, 128], F32, tag="warm", name="pw")
            for lt in (0,):
                ls = slice(lt * 512, (lt + 1) * 512)
                psk = kvp.tile([128, 512], F32, tag="psk", name="psk")
                psv = kvp.tile([128, 512], F32, tag="psv", name="psv")
                psq = [qf.tile([128, 512], F32, tag=f"psq{h}", name=f"psq{h}") for h in range(NH)]
                for k in range(DC):
                    st = k == 0
                    sp = k == DC - 1
                    nc.tensor.matmul(psk[:], lhsT=w_ch[k][:, QC:QC + HD], rhs=x_sb[k][:, ls], start=st, stop=sp)
                    nc.tensor.matmul(psv[:], lhsT=w_ch[k][:, QC + HD:], rhs=x_sb[k][:, ls], start=st, stop=sp)
                    for h in range(NH):
                        nc.tensor.matmul(
                            psq[h][:], lhsT=w_ch[k][:, h * 128:(h + 1) * 128], rhs=x_sb[k][:, ls],
                            start=st, stop=sp,
                        )
                    if k >= 2:  # ones_bf (DVE memset) is ready by ~7.6us
                        for _ in range(2):
                            nc.tensor.matmul(pw[:], lhsT=ones_bf[:], rhs=ones_bf[:], start=True, stop=True)
                nc.scalar.activation(kT[:, ls], psk[:], AF.Identity, bias=bk_sb[:, 0:1])
                nc.scalar.activation(vT[lt][:], psv[:], AF.Identity, bias=bv_sb[:, 0:1])
                for h in range(NH):
                    nc.scalar.activation(qT[h][:, ls], psq[h][:], AF.Identity, bias=bq_sb[:, h:h + 1])
        # l-tiles 1-3 K/V at full PE speed; each tile's V transposes slot
        # into the NEXT tile's matmul stream (their vT drain is long done by
        # then), copies on the DVE so the ACT stays clear.  A fresh
        # double-buffered pool (banks freed by qf/warmup) keeps tile lt+1
        # from waiting on tile lt's ACT drain.
        with (
            tc.tile_pool(name="kvp2", bufs=2, space="PSUM") as kvp2,
            tc.tile_pool(name="tpg", bufs=2, space="PSUM") as tpg,
        ):
            def transpose_vt(lt):
                for jj in range(4):
                    j = lt * 4 + jj
                    pt = tpg.tile([128, 128], BF16, tag="tp", name="tp")
                    nc.tensor.transpose(pt[:], vT[lt][:, jj * 128:(jj + 1) * 128], ident[:])
                    nc.vector.tensor_copy(vN[:, j * 128:(j + 1) * 128], pt[:])

            for lt in (1, 2, 3):
                ls = slice(lt * 512, (lt + 1) * 512)
                psk = kvp2.tile([128, 512], F32, tag="psk", name="psk")
                psv = kvp2.tile([128, 512], F32, tag="psv", name="psv")
                for k in range(DC):
                    st = k == 0
                    sp = k == DC - 1
                    nc.tensor.matmul(psk[:], lhsT=w_ch[k][:, QC:QC + HD], rhs=x_sb[k][:, ls], start=st, stop=sp)
                    nc.tensor.matmul(psv[:], lhsT=w_ch[k][:, QC + HD:], rhs=x_sb[k][:, ls], start=st, stop=sp)
                    if k == 8:
                        transpose_vt(lt - 1)
                nc.scalar.activation(kT[:, ls], psk[:], AF.Identity, bias=bk_sb[:, 0:1])
                nc.scalar.activation(vT[lt][:], psv[:], AF.Identity, bias=bv_sb[:, 0:1])
            transpose_vt(3)

    # ---------------- attention supersteps ----------------
    qp = ctx.enter_context(tc.tile_pool(name="qp", bufs=1, space="PSUM"))  # 2 tags = 2 banks
    sps = ctx.enter_context(tc.tile_pool(name="sps", bufs=2, space="PSUM"))  # 2 x [128,1024] = 4 banks
    avp = ctx.enter_context(tc.tile_pool(name="avp", bufs=1, space="PSUM"))  # [128,1024] = 2 banks
    attp = ctx.enter_context(tc.tile_pool(name="att", bufs=6))
    accp = ctx.enter_context(tc.tile_pool(name="acc", bufs=2))
    avsbp = ctx.enter_context(tc.tile_pool(name="avsb", bufs=2))
    finp = ctx.enter_context(tc.tile_pool(name="fin", bufs=3))

    def q_sweep_items(t, pair):
        """Generator of emission thunks: 32 matmuls (heads 2*pair, 2*pair+1
        over 16 d-chunks) + 2 ACT drains. Lazily allocates its 2 PSUM tiles
        on first next()."""
        ls = slice(t * 512, (t + 1) * 512)
        ps = [qp.tile([128, 512], F32, tag=f"psq{j}", name=f"psq{j}") for j in (0, 1)]
        for k in range(DC):
            st = k == 0
            sp = k == DC - 1
            for j in (0, 1):
                h = 2 * pair + j
                yield lambda k=k, j=j, h=h, st=st, sp=sp: nc.tensor.matmul(
                    ps[j][:],
                    lhsT=w_ch[k][:, h * 128:(h + 1) * 128],
                    rhs=x_sb[k][:, ls],
                    start=st,
                    stop=sp,
                )
        for j in (0, 1):
            h = 2 * pair + j
            yield lambda j=j, h=h: nc.scalar.activation(
                qT[h][:, ls], ps[j][:], AF.Identity, bias=bq_sb[:, h:h + 1]
            )
        # sentinel: marks this sweep fully consumed (checked when a pass's
        # hoisted scores are emitted)
        yield lambda t=t, pair=pair: done_sweeps.add(("q", t, pair))

    done_sweeps = set()
    tail = {"pending": None}

    def emit_tail(info, split=2):
        """Softmax tail of a finished pass: partition-reduce+replicate the
        DVE-summed denominator with two 128-wide ones-matmuls (into a
        rotating scores PSUM slot), fast-reciprocal + normalize per
        512-half, DMA out in `split` pieces per head."""
        acc, av_sb, t, hp, _psA = info
        psR = sps.tile([128, 1024], F32, tag="sps", name="psR")
        for j in (0, 1):
            nc.tensor.matmul(
                psR[:, j * 512:(j + 1) * 512],
                lhsT=ones_bf[:],
                rhs=acc[:, j * 512:(j + 1) * 512],
                start=True,
                stop=True,
            )
        ot = finp.tile([128, 1024], F32, tag="ot", name="ot")
        n = 512 // split
        for j in (0, 1):
            h = 2 * hp + j
            js = slice(j * 512, (j + 1) * 512)
            rinv = finp.tile([128, 512], F32, tag="rinv", name="rinv")
            nc.vector.reciprocal_approx_fast(rinv[:], psR[:, js])
            nc.vector.tensor_mul(ot[:, js], av_sb[:, js], rinv[:])
            for s in range(split):
                cs = slice(t * 512 + s * n, t * 512 + (s + 1) * n)
                nc.sync.dma_start(
                    out=outT[h * 128:(h + 1) * 128, cs],
                    in_=ot[:, j * 512 + s * n:j * 512 + (s + 1) * n],
                )

    def emit_final_tail(info):
        """Same as emit_tail but with the reciprocal/multiply/DMA chain cut
        into small staggered pieces: nothing runs after it, so its full
        latency is exposed and per-queue DMA bandwidth (~17GB/s) makes one
        big output transfer the kernel's last 8+us otherwise.  Multiplies
        read the AV PSUM directly — no SBUF drain on the critical path."""
        acc, _av_sb, t, hp, psA = info
        psR = sps.tile([128, 1024], F32, tag="sps", name="psR")
        for j in (0, 1):
            nc.tensor.matmul(
                psR[:, j * 512:(j + 1) * 512],
                lhsT=ones_bf[:],
                rhs=acc[:, j * 512:(j + 1) * 512],
                start=True,
                stop=True,
            )
        ot = finp.tile([128, 1024], F32, tag="ot", name="ot")
        for j in (0, 1):
            h = 2 * hp + j
            js = slice(j * 512, (j + 1) * 512)
            rinv = finp.tile([128, 512], F32, tag="rinv", name="rinv")
            nc.vector.reciprocal_approx_fast(rinv[:], psR[:, js])
            for q in range(4):  # [128,128] pieces: mul then DMA, staggered
                qs_ = slice(j * 512 + q * 128, j * 512 + (q + 1) * 128)
                nc.vector.tensor_mul(ot[:, qs_], psA[:, qs_], rinv[:, q * 128:(q + 1) * 128])
                cs = slice(t * 512 + q * 128, t * 512 + (q + 1) * 128)
                # spread the 8 final issues over three DGEs — serial issue
                # on one queue (~0.45us apiece) would stagger the last
                # transfers by ~3.6us
                eng = (nc.sync, nc.scalar, nc.gpsimd)[(2 * j + q) % 3]
                eng.dma_start(out=outT[h * 128:(h + 1) * 128, cs], in_=ot[:, qs_])

    # One flat filler stream, consumed earliest-deadline-first: Q slice
    # t+1's sweep A is only due before pass(t+1, 0) and sweep B before
    # pass(t+1, 1), so the 204 filler items spread over the first ~6.5
    # passes; build-time asserts verify every due-date.
    fill = itertools.chain(
        *[q_sweep_items(t, p) for t in (1, 2, 3) for p in (0, 1)]
    )

    def F(n=1):
        for _ in range(n):
            th = next(fill, None)
            if th is not None:
                th()

    class Pass:
        """Attention pass (lq slice t, head-pair hp). sc/av lazily allocate
        their pool tiles so the first two score blocks can be emitted from
        inside the PREVIOUS pass's tail (the ACT exp stream then crosses
        pass boundaries without a break)."""

        def __init__(self, t, hp):
            self.t, self.hp = t, hp
            self.qs = slice(t * 512, (t + 1) * 512)
            self.acc = None
            self.psA = None
            self.at_of = {}

        def sc(self, lk):
            assert self.t == 0 or ("q", self.t, self.hp) in done_sweeps, (
                f"Q sweep ({self.t},{self.hp}) not consumed before its scores"
            )
            ss = sps.tile([128, 1024], F32, tag="sps", name="sps")
            for j in (0, 1):
                nc.tensor.matmul(
                    ss[:, j * 512:(j + 1) * 512],
                    lhsT=kT[:, lk * 128:(lk + 1) * 128],
                    rhs=qT[2 * self.hp + j][:, self.qs],
                    start=True,
                    stop=True,
                )
            at = attp.tile([128, 1024], BF16, tag="att", name="att")
            nc.scalar.activation(at[:], ss[:], AF.Exp, scale=SCALE)
            # softmax denominator partials on the DVE (2x bf16 rate)
            if lk == 0:
                self.acc = accp.tile([128, 1024], BF16, tag="acc", name="acc")
                nc.vector.tensor_copy(self.acc[:], at[:])
            else:
                nc.vector.tensor_add(self.acc[:], self.acc[:], at[:])
            self.at_of[lk] = at

        def av(self, lk):
            if lk == 0:
                self.psA = avp.tile([128, 1024], F32, tag="av", name="av")
            for j in (0, 1):
                nc.tensor.matmul(
                    self.psA[:, j * 512:(j + 1) * 512],
                    lhsT=vN[:, lk * 128:(lk + 1) * 128],
                    rhs=self.at_of[lk][:, j * 512:(j + 1) * 512],
                    start=lk == 0,
                    stop=lk == LKT - 1,
                )

    passes = [Pass(t, hp) for t in range(NLT) for hp in (0, 1)]
    for i, p in enumerate(passes):
        nxt = passes[i + 1] if i + 1 < len(passes) else None
        # boundary: this pass's sc0/sc1 were already hoisted into the
        # previous pass's tail; sc2 goes first here so the ACT's exp
        # stream crosses the boundary with no bubble, then the previous
        # pass's softmax tail
        if i == 0:
            p.sc(0)
            F(2)
            p.sc(1)
            F(2)
        p.sc(2)
        F(3)
        if tail["pending"] is not None:
            emit_tail(tail["pending"])
            tail["pending"] = None
        F(1)
        # steady state: AV trails scores by 3 blocks
        for lk in range(3, LKT):
            p.sc(lk)
            F(1)
            p.av(lk - 3)
            F(1)
        # hoist the next pass's first two score blocks into this tail
        if nxt is not None:
            nxt.sc(0)
        p.av(LKT - 3)
        if nxt is not None:
            nxt.sc(1)
        p.av(LKT - 2)
        F(1)
        p.av(LKT - 1)
        # drain the AV accumulator to SBUF on the DVE (frees its PSUM banks
        # for the next pass, off the ACT so the exp stream never waits).
        # The final pass is normalized straight out of PSUM instead.
        if nxt is None:
            av_sb = None
        else:
            av_sb = avsbp.tile([128, 1024], F32, tag="avsb", name="avsb")
            nc.vector.tensor_copy(av_sb[:], p.psA[:])
        tail["pending"] = (p.acc, av_sb, p.t, p.hp, p.psA)
    assert next(fill, None) is None, "filler stream not fully consumed"
    emit_final_tail(tail["pending"])
    tail["pending"] = None


_NC_CACHE = None


def build_nc():
    global _NC_CACHE
    if _NC_CACHE is not None:
        return _NC_CACHE
    nc = bacc.Bacc("TRN2", target_bir_lowering=False, debug=False)
    xT = nc.dram_tensor("xT", [D, L], F16, kind="ExternalInput").ap()
    wT = nc.dram_tensor("wT", [D, QC + 2 * HD], F16, kind="ExternalInput").ap()
    bq = nc.dram_tensor("bq", [128, NH], F32, kind="ExternalInput").ap()
    bk = nc.dram_tensor("bk", [128, 1], F32, kind="ExternalInput").ap()
    bv = nc.dram_tensor("bv", [128, 1], F32, kind="ExternalInput").ap()
    outT = nc.dram_tensor("outT", [QC, L], F32, kind="ExternalOutput").ap()
    with tile.TileContext(nc) as tc, ExitStack() as ctx:
        build_kernel(ctx, tc, xT, wT, bq, bk, bv, outT)
    nc.compile()
    _NC_CACHE = nc
    return nc


def make_in_maps(x, Wq_w, Wq_b, Wk_w, Wk_b, Wv_w, Wv_b):
    """Host-side sharding/relayout. Returns one input map per core."""
    x = np.asarray(x, dtype=np.float32)
    Wq_w = np.asarray(Wq_w, dtype=np.float32)
    Wq_b = np.asarray(Wq_b, dtype=np.float32)
    Wk_w = np.asarray(Wk_w, dtype=np.float32)
    Wk_b = np.asarray(Wk_b, dtype=np.float32)
    Wv_w = np.asarray(Wv_w, dtype=np.float32)
    Wv_b = np.asarray(Wv_b, dtype=np.float32)

    xTs = [np.ascontiguousarray(x[b].T).astype(np.float16) for b in range(B)]
    wkvT = np.concatenate([Wk_w.T, Wv_w.T], axis=1)  # [D, 256]
    bk = np.ascontiguousarray(Wk_b.reshape(128, 1))
    bv = np.ascontiguousarray(Wv_b.reshape(128, 1))
    in_maps = []
    for c in range(N_CORES):
        b, g = divmod(c, B * 2)  # b = c // 4, g = c % 4
        # one contiguous [D, 768] weight tensor: [Wq_g | Wk | Wv].T
        wT_g = np.ascontiguousarray(
            np.concatenate([Wq_w[g * QC:(g + 1) * QC, :].T, wkvT], axis=1)
        ).astype(np.float16)
        bq_g = np.ascontiguousarray(Wq_b[g * QC:(g + 1) * QC].reshape(NH, 128).T)
        in_maps.append(
            {
                "xT": xTs[b],
                "wT": wT_g,
                "bq": bq_g,
                "bk": bk,
                "bv": bv,
            }
        )
    return in_maps


def assemble_output(results):
    out = np.empty((B, L, D), dtype=np.float32)
    for c in range(N_CORES):
        b, g = divmod(c, B * 2)
        out[b, :, g * QC:(g + 1) * QC] = results[c]["outT"].T
    return out


def kernel(**inputs) -> np.ndarray:
    nc = build_nc()
    in_maps = make_in_maps(**inputs)
    res = run_bass_kernel_spmd(nc, in_maps, core_ids=list(range(N_CORES)))
    return assemble_output(res.results)
